# revision 20
# baseline (speedup 1.0000x reference)
"""Trainium2 Bass kernel for nn_Decoder_68289980006849 (3-layer transformer decoder).

Strategy: data-parallel over batch (B=8) across 8 NeuronCores; zero collectives.
Per core, the full decoder runs in "T-layout" [feature(partitions), token(free)]
with float32r matmuls (full PE rate, ~tf32 precision):

  - embeddings gathered on host (input sharding); device applies *sqrt(D) + pe
  - LayerNorm: mean/mean-of-square via ones-matmul on PE -> stats arrive
    pre-broadcast as [128, 512] PSUM tiles; apply with 2 DVE ops/chunk
  - attention: scores computed transposed (scores_T[tk, tq]) so the softmaxed
    matrix feeds the context matmul directly (no transposes); denominators come
    free from ones-columns appended to V; exp on ACT with fused 1/sqrt(dk);
    causal masking via column-slicing + one diagonal-block multiply
  - normalization by softmax denominator fused into the PSUM eviction
  - FFN processed in 4 f-quarters (SBUF); residuals fused into evictions
    (incl. the reference quirk x = cross_out + ffn_out); cross-attn output
    parked in DRAM between its producer and the FFN2 eviction
  - weights pre-transposed and pre-packed on host into exact SBUF tile images
    so every weight DMA is one contiguous 2MB transfer

Self-contained: only stdlib + numpy + the concourse/bass stack on PYTHONPATH.
"""

import os
import numpy as np

import concourse.bass as bass
import concourse.tile as tile
from concourse import bacc, mybir
from concourse.masks import make_identity

# ---- problem constants (hardcoded per contract) ----
B, LD, LE = 8, 512, 512
D, H, DK, F, L, V = 1024, 16, 64, 4096, 3, 32000
M = LD                      # tokens per core
DCH = D // 128              # 8 d-model chunks
FCH = F // 128              # 32 ffn chunks
MCH = M // 128              # 4 token chunks
SQRT_D = 32.0
INV_SQRT_DK = 0.125
EPS = 1e-5
NONES = 32                  # ones-columns appended to V (denominator rows)

P = 128
N = 512
F32 = mybir.dt.float32
F32R = mybir.dt.float32r
AF = mybir.ActivationFunctionType
ALU = mybir.AluOpType

_CACHE = {}


# ----------------------------------------------------------------------------
# Bass program (identical on all 8 cores; data differs via in_maps)
# ----------------------------------------------------------------------------

def _build_nc():
    nc = bacc.Bacc("TRN2", target_bir_lowering=False, debug=False,
                   enable_asserts=False, num_devices=8)

    # inputs (per core)
    x0p = nc.dram_tensor("x0p", [P, DCH, N], F32R, kind="ExternalInput").ap()
    pep = nc.dram_tensor("pep", [P, DCH, N], F32, kind="ExternalInput").ap()
    encp = nc.dram_tensor("encp", [P, DCH, N], F32R, kind="ExternalInput").ap()
    # projection weights, packed: [l, a, i(q,k,v,o), g, 128, 8, 512]
    wp = nc.dram_tensor("wp", [L, 2, 4, 2, P, DCH, N], F32R, kind="ExternalInput").ap()
    w1p = nc.dram_tensor("w1p", [L, 8, P, DCH, N], F32R, kind="ExternalInput").ap()
    # FFN2 weights packed per f-eighth: [128, j(8), ko(4), 128]
    w2p = nc.dram_tensor("w2p", [L, 8, P, DCH, 4, P], F32R, kind="ExternalInput").ap()
    causal = nc.dram_tensor("causal", [P, P], F32, kind="ExternalInput").ap()
    jscale = nc.dram_tensor("jscale", [P, P], F32R, kind="ExternalInput").ap()
    vones = nc.dram_tensor("vones", [P, MCH, H, NONES], F32R, kind="ExternalInput").ap()
    cpark = nc.dram_tensor("cpark", [P, DCH, N], F32R, kind="Internal").ap()
    out = nc.dram_tensor("out", [M, D], F32, kind="ExternalOutput").ap()

    with tile.TileContext(nc) as tc:
        with tc.tile_pool(name="res", bufs=1) as res, \
             tc.tile_pool(name="wpool", bufs=2) as wpool, \
             tc.tile_pool(name="spool", bufs=2) as spool, \
             tc.tile_pool(name="psum", bufs=1, space="PSUM") as psum:

            # ---- resident tiles ----
            xT = res.tile([P, DCH, N], F32R)        # residual stream
            encT = res.tile([P, DCH, N], F32R)      # encoder output (transposed)
            Vst = res.tile([P, MCH, H, 64 + NONES], F32R)  # V + ones columns
            Jsc = res.tile([P, P], F32R)            # all-(1/D) for LN stats
            c01 = res.tile([P, P], F32)             # causal diagonal 0/1 keep-mask
            ident = res.tile([P, P], F32)
            epsc = res.tile([P, 1], F32)

            nc.sync.dma_start(encT[:], encp)
            nc.sync.dma_start(Jsc[:], jscale)
            nc.sync.dma_start(c01[:], causal)
            nc.sync.dma_start(Vst[:, :, :, 64:64 + NONES], vones)
            make_identity(nc, ident[:])
            nc.vector.memset(epsc[:], EPS)

            # ---- x = x0 * sqrt(D) + pe (short-lived staging pool) ----
            with tc.tile_pool(name="init", bufs=1) as init:
                x0 = init.tile([P, DCH, N], F32R, tag="x0")
                pe = init.tile([P, DCH, N], F32, tag="pe")
                nc.sync.dma_start(x0[:], x0p)
                nc.sync.dma_start(pe[:], pep)
                for c in range(DCH):
                    nc.vector.scalar_tensor_tensor(
                        xT[:, c], x0[:, c].bitcast(F32), SQRT_D, pe[:, c],
                        ALU.mult, ALU.add)

            work = tc.alloc_tile_pool(name="work", bufs=1)
            hT = work.tile([P, DCH, N], F32R, tag="hT")
            ctxT = work.tile([P, DCH, N], F32R, tag="ctxT")
            qT = work.tile([P, DCH, N], F32R, tag="qT")
            kT = work.tile([P, DCH, N], F32R, tag="kT")


            # ---- helpers ----
            def layernorm(dst):
                """dst[:] = layernorm(xT) along features."""
                mean_ps = psum.tile([P, N], F32, tag="sc", bufs=4, name="mean_ps")
                msq_ps = psum.tile([P, N], F32, tag="sc", bufs=4, name="msq_ps")
                for c in range(DCH):
                    sq = spool.tile([P, N], F32R, tag="sq", bufs=1, name="sq")
                    nc.scalar.activation(sq[:], xT[:, c].bitcast(F32), AF.Square)
                    nc.tensor.matmul(mean_ps[:], Jsc[:], xT[:, c],
                                     start=(c == 0), stop=(c == DCH - 1))
                    nc.tensor.matmul(msq_ps[:], Jsc[:], sq[:],
                                     start=(c == 0), stop=(c == DCH - 1))
                m2 = spool.tile([P, N], F32, tag="stt", name="m2")
                nc.scalar.activation(m2[:], mean_ps[:], AF.Square)
                var = spool.tile([P, N], F32, tag="stt", name="var")
                nc.vector.tensor_tensor(var[:], msq_ps[:], m2[:], op=ALU.subtract)
                sd = spool.tile([P, N], F32, tag="stt", name="sd")
                nc.scalar.activation(sd[:], var[:], AF.Sqrt, bias=epsc[:])
                rstd = spool.tile([P, N], F32, tag="stt", name="rstd")
                nc.vector.reciprocal_approx_fast(rstd[:], sd[:])
                for c in range(DCH):
                    cen = spool.tile([P, N], F32, tag="cen", name="cen")
                    nc.vector.tensor_tensor(cen[:], xT[:, c].bitcast(F32), mean_ps[:],
                                            op=ALU.subtract)
                    nc.vector.tensor_tensor(dst[:, c], cen[:], rstd[:], op=ALU.mult)

            def proj(rhs, w_groups, evict):
                """out[j] = sum_k W[k, j-chunk].T @ rhs[k]; W streamed in 2MB tiles.
                The DMA lands in column halves so the first out-chunks' matmuls
                can start as soon as half the tile has arrived."""
                for g in range(2):
                    wt = wpool.tile([P, DCH, N], F32R, tag="wt", name="wt")
                    nc.sync.dma_start(wt[:, :, 0:N // 2], w_groups[g][:, :, 0:N // 2])
                    nc.sync.dma_start(wt[:, :, N // 2:], w_groups[g][:, :, N // 2:])
                    for jj in range(4):
                        j = g * 4 + jj
                        ps = psum.tile([P, N], F32, tag="mm", bufs=2, name="ps_mm")
                        for k in range(DCH):
                            nc.tensor.matmul(ps[:], wt[:, k, jj * P:(jj + 1) * P],
                                             rhs[:, k], start=(k == 0),
                                             stop=(k == DCH - 1))
                        evict(j, ps)

            def v_proj(kv_rhs, w_groups):
                """V_nat[tk, dv] -> Vst[:, t, h, :64] slices."""
                for g in range(2):  # dv halves (heads g*8..g*8+7)
                    wt = wpool.tile([P, DCH, N], F32R, tag="wt", name="wt")
                    nc.sync.dma_start(wt[:], w_groups[g])
                    for t in range(MCH):
                        ps = psum.tile([P, N], F32, tag="mm", bufs=2, name="ps_v")
                        for k in range(DCH):
                            nc.tensor.matmul(ps[:], kv_rhs[:, k, t * P:(t + 1) * P],
                                             wt[:, k], start=(k == 0),
                                             stop=(k == DCH - 1))
                        nc.scalar.activation(
                            Vst[:, t, g * 8:(g + 1) * 8, 0:64],
                            ps[:].rearrange("p (h d) -> p h d", d=64), AF.Copy)

            def attention(q_t, k_t, is_self, wo_groups, out_evict):
                """softmax((q_t.T k_t)/sqrt(dk)) @ V via transposed scores.

                Head pairs are issued adjacently: the even head's K=64 score
                matmuls occupy PE rows 0:64, the odd head's rows 64:128
                (tile_position auto-derived from base_partition), so both run
                concurrently in the array."""
                for hp in range(H // 2):
                    exps = {0: [], 1: []}
                    for c in range(MCH):
                        cs = c * P if is_self else 0
                        for half in (0, 1):
                            off = 64 * half
                            sc = psum.tile([P, N], F32, tag="sc", bufs=4, name="sc")
                            nc.tensor.matmul(sc[:, cs:],
                                             k_t[off:off + 64, hp, c * P:(c + 1) * P],
                                             q_t[off:off + 64, hp, cs:],
                                             start=True, stop=True)
                            ex = spool.tile([P, N], F32R, tag="exp", bufs=8, name="ex")
                            nc.scalar.activation(ex[:, cs:], sc[:, cs:], AF.Exp,
                                                 scale=INV_SQRT_DK)
                            if is_self:
                                nc.vector.tensor_tensor(
                                    ex[:, c * P:(c + 1) * P],
                                    ex[:, c * P:(c + 1) * P].bitcast(F32), c01[:],
                                    op=ALU.mult)
                            exps[half].append(ex)
                    for half in (0, 1):
                        h, off = 2 * hp + half, 64 * half
                        ctx = psum.tile([P, N], F32, tag="ctx", bufs=2, name="ctx")
                        for c in range(MCH):
                            cs = c * P if is_self else 0
                            nc.tensor.matmul(ctx[0:64 + NONES, cs:], Vst[:, c, h, :],
                                             exps[half][c][:, cs:], start=(c == 0),
                                             stop=(c == MCH - 1),
                                             skip_group_check=True)
                        den = spool.tile([NONES, N], F32, tag="den", bufs=1, name="den")
                        nc.scalar.activation(den[:], ctx[64:64 + NONES, :], AF.Copy)
                        rec = spool.tile([NONES, N], F32, tag="rec", name="rec")
                        nc.vector.reciprocal_approx_fast(rec[:], den[:])
                        nc.vector.tensor_tensor(ctxT[off:off + 32, hp, :], ctx[0:32, :],
                                                rec[:], op=ALU.mult)
                        nc.vector.tensor_tensor(ctxT[off + 32:off + 64, hp, :],
                                                ctx[32:64, :], rec[:], op=ALU.mult)
                proj(ctxT, wo_groups, out_evict)

            for l in range(L):
                def wgrp(a, i):
                    return [wp[l, a, i, g] for g in range(2)]

                # ---- self-attention ----
                layernorm(hT)

                def ev_to(dst):
                    return lambda j, ps: nc.scalar.activation(dst[:, j], ps[:], AF.Copy)
                proj(hT, wgrp(0, 0), ev_to(qT))
                proj(hT, wgrp(0, 1), ev_to(kT))
                v_proj(hT, wgrp(0, 2))

                def ev_self_o(j, ps):
                    nc.vector.tensor_tensor(xT[:, j], ps[:], xT[:, j].bitcast(F32),
                                            op=ALU.add)
                attention(qT, kT, True, wgrp(0, 3), ev_self_o)

                # ---- cross-attention ----
                layernorm(hT)
                proj(hT, wgrp(1, 0), ev_to(qT))
                proj(encT, wgrp(1, 1), ev_to(kT))
                v_proj(encT, wgrp(1, 2))

                def ev_cross_o(j, ps):
                    ct = spool.tile([P, N], F32R, tag="ctile", bufs=1, name="ct")
                    nc.scalar.activation(ct[:], ps[:], AF.Copy)
                    nc.sync.dma_start(cpark[:, j], ct[:])
                    nc.vector.tensor_tensor(xT[:, j], ps[:], xT[:, j].bitcast(F32),
                                            op=ALU.add)
                attention(qT, kT, False, wgrp(1, 3), ev_cross_o)

                # ---- FFN (8 f-eighths, double-buffered u) ----
                layernorm(hT)
                for e8 in range(8):
                    # FFN1: produce eighth e8 of u (4 f-chunks)
                    wt = wpool.tile([P, DCH, N], F32R, tag="wt", name="wt")
                    nc.sync.dma_start(wt[:, :, 0:N // 2], w1p[l, e8][:, :, 0:N // 2])
                    nc.sync.dma_start(wt[:, :, N // 2:], w1p[l, e8][:, :, N // 2:])
                    u8 = work.tile([P, 4, N], F32R, tag="uT", bufs=2, name="u8")
                    for jj in range(4):
                        ps = psum.tile([P, N], F32, tag="mm", bufs=2, name="ps_f1")
                        for k in range(DCH):
                            nc.tensor.matmul(ps[:], wt[:, k, jj * P:(jj + 1) * P],
                                             hT[:, k], start=(k == 0),
                                             stop=(k == DCH - 1))
                        nc.scalar.activation(u8[:, jj], ps[:], AF.Relu)
                    # FFN2: partial contraction over this eighth (all 8 e-chunks)
                    wt2 = wpool.tile([P, DCH, 4, P], F32R, tag="wt", name="wt2")
                    nc.sync.dma_start(wt2[:], w2p[l, e8])
                    for j in range(DCH):
                        ps = psum.tile([P, N], F32, tag="mm", bufs=2, name="ps_f2")
                        for k in range(4):
                            nc.tensor.matmul(ps[:], wt2[:, j, k], u8[:, k],
                                             start=(k == 0), stop=(k == 3))
                        if e8 == 0:
                            # x = cross_out + ffn_part0 (reference residual quirk)
                            ct = spool.tile([P, N], F32R, tag="ctile", bufs=1, name="ct2")
                            nc.sync.dma_start(ct[:], cpark[:, j])
                            nc.vector.tensor_tensor(xT[:, j], ps[:],
                                                    ct[:].bitcast(F32), op=ALU.add)
                        else:
                            nc.vector.tensor_tensor(xT[:, j], ps[:],
                                                    xT[:, j].bitcast(F32), op=ALU.add)

            # ---- transpose to [tokens, features] and store ----
            # j-outer so each feature chunk's transposes overlap the last
            # layer's remaining FFN2 evictions (xT[:, j] finalizes per j).
            for j in range(DCH):
                for m in range(MCH):
                    pst = psum.tile([P, N], F32, tag="sc", bufs=4, name="pst")
                    nc.tensor.transpose(pst[:, 0:P],
                                        xT[:, j, m * P:(m + 1) * P].bitcast(F32),
                                        ident[:])
                    tsb = spool.tile([P, P], F32, tag="osb", bufs=2, name="tsb")
                    nc.scalar.activation(tsb[:], pst[:, 0:P], AF.Copy)
                    nc.sync.dma_start(
                        out[m * P:(m + 1) * P, j * P:(j + 1) * P], tsb[:])

            work.release()

    nc.compile()
    return nc


# ----------------------------------------------------------------------------
# host-side packing
# ----------------------------------------------------------------------------

def _pack_T(aT):
    """[1024, C] (feature-major) -> tile image [128, 8, C]."""
    d, c = aT.shape
    return np.ascontiguousarray(aT.reshape(DCH, P, c).transpose(1, 0, 2))


def _pack_proj(w):
    """w [Dout, Din] (as in y = x @ w.T) -> [2, 128, 8, 512] group tile images."""
    wT = w.T  # [Din, Dout]
    return np.stack([_pack_T(wT[:, g * N:(g + 1) * N]) for g in range(2)])


def _prep(inputs):
    dec_inputs = np.asarray(inputs["dec_inputs"])
    self_mask = np.asarray(inputs["self_mask"])
    enc_output = np.asarray(inputs["enc_output"], dtype=np.float32)
    encoder_mask = np.asarray(inputs["encoder_mask"])
    embed = np.asarray(inputs["embed"], dtype=np.float32)
    pe = np.asarray(inputs["pe"], dtype=np.float32)
    wq, wk, wv, wo = (np.asarray(inputs[k], np.float32) for k in ("wq", "wk", "wv", "wo"))
    w1, w2 = np.asarray(inputs["ffn_w1"], np.float32), np.asarray(inputs["ffn_w2"], np.float32)

    # structural assumptions baked into the kernel
    causal_ref = np.triu(np.ones((LD, LD), bool), k=1)
    assert all(np.array_equal(self_mask[b], causal_ref) for b in range(B)), \
        "kernel assumes causal self mask"
    assert not encoder_mask.any(), "kernel assumes no encoder mask"
    for k in ("bq", "bk", "bv", "bo", "ffn_b1", "ffn_b2", "ln_b"):
        assert not np.asarray(inputs[k]).any(), f"kernel assumes zero {k}"
    assert np.all(np.asarray(inputs["ln_g"]) == 1.0), "kernel assumes unit ln gains"

    # shared (weight) arrays
    wp = np.empty((L, 2, 4, 2, P, DCH, N), np.float32)
    for l in range(L):
        for a in range(2):
            for i, w in enumerate((wq, wk, wv, wo)):
                wp[l, a, i] = _pack_proj(w[l, a])
    w1p = np.empty((L, 8, P, DCH, N), np.float32)
    w2p = np.empty((L, 8, P, DCH, 4, P), np.float32)
    for l in range(L):
        w1T = w1[l].T  # [1024, 4096]
        for g in range(8):
            w1p[l, g] = _pack_T(w1T[:, g * N:(g + 1) * N])
        w2T = w2[l].T  # [4096, 1024]
        # [e8, ko, ki, j, e] -> [e8, ki, j, ko, e]
        blk = w2T.reshape(8, 4, P, DCH, P)
        w2p[l] = np.ascontiguousarray(blk.transpose(0, 2, 3, 1, 4))

    pep = _pack_T(pe.T)
    causal01 = (~causal_ref[:P, :P]).astype(np.float32).T.copy()  # keep[tk, tq]
    jscale = np.full((P, P), 1.0 / D, np.float32)
    vones = np.ones((P, MCH, H, NONES), np.float32)

    shared = dict(wp=wp, w1p=w1p, w2p=w2p, pep=pep, causal=causal01,
                  jscale=jscale, vones=vones)
    in_maps = []
    for b in range(B):
        x0 = embed[dec_inputs[b]]          # [512, 1024]
        m = dict(shared)
        m["x0p"] = _pack_T(np.ascontiguousarray(x0.T))
        m["encp"] = _pack_T(np.ascontiguousarray(enc_output[b].T))
        in_maps.append(m)
    return in_maps


def kernel(**inputs):
    if "nc" not in _CACHE:
        _CACHE["nc"] = _build_nc()
    nc = _CACHE["nc"]
    in_maps = _prep(inputs)

    from concourse import bass_utils
    trace = bool(int(os.environ.get("DECODER_TRACE", "0")))
    res = bass_utils.run_bass_kernel_spmd(
        nc, in_maps, core_ids=list(range(B)), trace=trace)
    _CACHE["last_result"] = res
    return np.stack([res.results[b]["out"] for b in range(B)]).astype(np.float32)


# revision 22
# speedup vs baseline: 1.0381x; 1.0381x over previous
"""Trainium2 Bass kernel for nn_Decoder_68289980006849 (3-layer transformer decoder).

Strategy: data-parallel over batch (B=8) across 8 NeuronCores; zero collectives.
Per core, the full decoder runs in "T-layout" [feature(partitions), token(free)]
with float32r matmuls (full PE rate, ~tf32 precision):

  - embeddings gathered on host (input sharding); device applies *sqrt(D) + pe
  - LayerNorm: mean/mean-of-square via ones-matmul on PE -> stats arrive
    pre-broadcast as [128, 512] PSUM tiles; apply with 2 DVE ops/chunk
  - attention: scores computed transposed (scores_T[tk, tq]) so the softmaxed
    matrix feeds the context matmul directly (no transposes); denominators come
    free from ones-columns appended to V; exp on ACT with fused 1/sqrt(dk);
    causal masking via column-slicing + one diagonal-block multiply
  - normalization by softmax denominator fused into the PSUM eviction
  - FFN processed in 4 f-quarters (SBUF); residuals fused into evictions
    (incl. the reference quirk x = cross_out + ffn_out); cross-attn output
    parked in DRAM between its producer and the FFN2 eviction
  - weights pre-transposed and pre-packed on host into exact SBUF tile images
    so every weight DMA is one contiguous 2MB transfer

Self-contained: only stdlib + numpy + the concourse/bass stack on PYTHONPATH.
"""

import os
import numpy as np

import concourse.bass as bass
import concourse.tile as tile
from concourse import bacc, mybir
from concourse.masks import make_identity

# ---- problem constants (hardcoded per contract) ----
B, LD, LE = 8, 512, 512
D, H, DK, F, L, V = 1024, 16, 64, 4096, 3, 32000
M = LD                      # tokens per core
DCH = D // 128              # 8 d-model chunks
FCH = F // 128              # 32 ffn chunks
MCH = M // 128              # 4 token chunks
SQRT_D = 32.0
INV_SQRT_DK = 0.125
EPS = 1e-5
NONES = 32                  # ones-columns appended to V (denominator rows)

P = 128
N = 512
F32 = mybir.dt.float32
F32R = mybir.dt.float32r
AF = mybir.ActivationFunctionType
ALU = mybir.AluOpType

_CACHE = {}


# ----------------------------------------------------------------------------
# Bass program (identical on all 8 cores; data differs via in_maps)
# ----------------------------------------------------------------------------

def _build_nc():
    nc = bacc.Bacc("TRN2", target_bir_lowering=False, debug=False,
                   enable_asserts=False, num_devices=8)

    # inputs (per core)
    x0p = nc.dram_tensor("x0p", [P, DCH, N], F32R, kind="ExternalInput").ap()
    pep = nc.dram_tensor("pep", [P, DCH, N], F32, kind="ExternalInput").ap()
    encp = nc.dram_tensor("encp", [P, DCH, N], F32R, kind="ExternalInput").ap()
    # projection weights, packed: [l, a, i(q,k,v,o), g, 128, 8, 512]
    wp = nc.dram_tensor("wp", [L, 2, 4, 2, P, DCH, N], F32R, kind="ExternalInput").ap()
    w1p = nc.dram_tensor("w1p", [L, 8, P, DCH, N], F32R, kind="ExternalInput").ap()
    # FFN2 weights packed per f-eighth: [128, j(8), ko(4), 128]
    w2p = nc.dram_tensor("w2p", [L, 8, P, DCH, 4, P], F32R, kind="ExternalInput").ap()
    causal = nc.dram_tensor("causal", [P, P], F32, kind="ExternalInput").ap()
    jscale = nc.dram_tensor("jscale", [P, P], F32R, kind="ExternalInput").ap()
    vones = nc.dram_tensor("vones", [P, MCH, H, NONES], F32R, kind="ExternalInput").ap()
    cpark = nc.dram_tensor("cpark", [P, DCH, N], F32R, kind="Internal").ap()
    out = nc.dram_tensor("out", [M, D], F32, kind="ExternalOutput").ap()

    with tile.TileContext(nc) as tc:
        with tc.tile_pool(name="res", bufs=1) as res, \
             tc.tile_pool(name="wpool", bufs=2) as wpool, \
             tc.tile_pool(name="spool", bufs=2) as spool, \
             tc.tile_pool(name="psum", bufs=1, space="PSUM") as psum:

            # ---- resident tiles ----
            xT = res.tile([P, DCH, N], F32R)        # residual stream
            encT = res.tile([P, DCH, N], F32R)      # encoder output (transposed)
            Vst = res.tile([P, MCH, H, 64 + NONES], F32R)  # V + ones columns
            Jsc = res.tile([P, P], F32R)            # all-(1/D) for LN stats
            c01 = res.tile([P, P], F32)             # causal diagonal 0/1 keep-mask
            ident = res.tile([P, P], F32)
            epsc = res.tile([P, 1], F32)

            nc.sync.dma_start(encT[:], encp)
            nc.sync.dma_start(Jsc[:], jscale)
            nc.sync.dma_start(c01[:], causal)
            nc.sync.dma_start(Vst[:, :, :, 64:64 + NONES], vones)
            make_identity(nc, ident[:])
            nc.vector.memset(epsc[:], EPS)

            # ---- x = x0 * sqrt(D) + pe (short-lived staging pool) ----
            with tc.tile_pool(name="init", bufs=1) as init:
                x0 = init.tile([P, DCH, N], F32R, tag="x0")
                pe = init.tile([P, DCH, N], F32, tag="pe")
                nc.sync.dma_start(x0[:], x0p)
                nc.sync.dma_start(pe[:], pep)
                for c in range(DCH):
                    nc.vector.scalar_tensor_tensor(
                        xT[:, c], x0[:, c].bitcast(F32), SQRT_D, pe[:, c],
                        ALU.mult, ALU.add)

            work = tc.alloc_tile_pool(name="work", bufs=1)
            hT = work.tile([P, DCH, N], F32R, tag="hT")
            ctxT = work.tile([P, DCH, N], F32R, tag="ctxT")
            qT = work.tile([P, DCH, N], F32R, tag="qT")
            kT = work.tile([P, DCH, N], F32R, tag="kT")


            # ---- helpers ----
            def layernorm(dst):
                """dst[:] = layernorm(xT) along features."""
                mean_ps = psum.tile([P, N], F32, tag="sc", bufs=4, name="mean_ps")
                msq_ps = psum.tile([P, N], F32, tag="sc", bufs=4, name="msq_ps")
                for c in range(DCH):
                    sq = spool.tile([P, N], F32R, tag="sq", bufs=2, name="sq")
                    nc.scalar.activation(sq[:], xT[:, c].bitcast(F32), AF.Square)
                    nc.tensor.matmul(mean_ps[:], Jsc[:], xT[:, c],
                                     start=(c == 0), stop=(c == DCH - 1))
                    nc.tensor.matmul(msq_ps[:], Jsc[:], sq[:],
                                     start=(c == 0), stop=(c == DCH - 1))
                m2 = spool.tile([P, N], F32, tag="stt", name="m2")
                nc.scalar.activation(m2[:], mean_ps[:], AF.Square)
                var = spool.tile([P, N], F32, tag="stt", name="var")
                nc.vector.tensor_tensor(var[:], msq_ps[:], m2[:], op=ALU.subtract)
                sd = spool.tile([P, N], F32, tag="stt", name="sd")
                nc.scalar.activation(sd[:], var[:], AF.Sqrt, bias=epsc[:])
                rstd = spool.tile([P, N], F32, tag="stt", name="rstd")
                nc.vector.reciprocal_approx_fast(rstd[:], sd[:])
                for c in range(DCH):
                    cen = spool.tile([P, N], F32, tag="cen", name="cen")
                    nc.vector.tensor_tensor(cen[:], xT[:, c].bitcast(F32), mean_ps[:],
                                            op=ALU.subtract)
                    nc.vector.tensor_tensor(dst[:, c], cen[:], rstd[:], op=ALU.mult)

            def proj(rhs, w_groups, evict):
                """out[j] = sum_k W[k, j-chunk].T @ rhs[k]; W streamed in 2MB tiles.
                The DMA lands in column halves so the first out-chunks' matmuls
                can start as soon as half the tile has arrived."""
                for g in range(2):
                    wt = wpool.tile([P, DCH, N], F32R, tag="wt", name="wt")
                    nc.sync.dma_start(wt[:, :, 0:N // 2], w_groups[g][:, :, 0:N // 2])
                    nc.sync.dma_start(wt[:, :, N // 2:], w_groups[g][:, :, N // 2:])
                    for jj in range(4):
                        j = g * 4 + jj
                        ps = psum.tile([P, N], F32, tag="mm", bufs=2, name="ps_mm")
                        for k in range(DCH):
                            nc.tensor.matmul(ps[:], wt[:, k, jj * P:(jj + 1) * P],
                                             rhs[:, k], start=(k == 0),
                                             stop=(k == DCH - 1))
                        evict(j, ps)

            def v_proj(kv_rhs, w_groups):
                """V_nat[tk, dv] -> Vst[:, t, h, :64] slices."""
                for g in range(2):  # dv halves (heads g*8..g*8+7)
                    wt = wpool.tile([P, DCH, N], F32R, tag="wt", name="wt")
                    nc.sync.dma_start(wt[:], w_groups[g])
                    for t in range(MCH):
                        ps = psum.tile([P, N], F32, tag="mm", bufs=2, name="ps_v")
                        for k in range(DCH):
                            nc.tensor.matmul(ps[:], kv_rhs[:, k, t * P:(t + 1) * P],
                                             wt[:, k], start=(k == 0),
                                             stop=(k == DCH - 1))
                        nc.scalar.activation(
                            Vst[:, t, g * 8:(g + 1) * 8, 0:64],
                            ps[:].rearrange("p (h d) -> p h d", d=64), AF.Copy)

            def attention(q_t, k_t, is_self, wo_groups, out_evict):
                """softmax((q_t.T k_t)/sqrt(dk)) @ V via transposed scores.

                Head pairs are issued adjacently: the even head's K=64 score
                matmuls occupy PE rows 0:64, the odd head's rows 64:128
                (tile_position auto-derived from base_partition), so both run
                concurrently in the array."""
                for h in range(H):
                    hc, off = h // 2, 64 * (h % 2)
                    exps = []
                    for c in range(MCH):
                        cs = c * P if is_self else 0
                        sc = psum.tile([P, N], F32, tag="sc", bufs=4, name="sc")
                        nc.tensor.matmul(sc[:, cs:],
                                         k_t[off:off + 64, hc, c * P:(c + 1) * P],
                                         q_t[off:off + 64, hc, cs:],
                                         start=True, stop=True)
                        ex = spool.tile([P, N], F32R, tag="exp", bufs=5, name="ex")
                        nc.scalar.activation(ex[:, cs:], sc[:, cs:], AF.Exp,
                                             scale=INV_SQRT_DK)
                        if is_self:
                            nc.vector.tensor_tensor(
                                ex[:, c * P:(c + 1) * P],
                                ex[:, c * P:(c + 1) * P].bitcast(F32), c01[:],
                                op=ALU.mult)
                        exps.append(ex)
                    ctx = psum.tile([P, N], F32, tag="ctx", bufs=2, name="ctx")
                    for c in range(MCH):
                        cs = c * P if is_self else 0
                        nc.tensor.matmul(ctx[0:64 + NONES, cs:], Vst[:, c, h, :],
                                         exps[c][:, cs:], start=(c == 0),
                                         stop=(c == MCH - 1), skip_group_check=True)
                    den = spool.tile([NONES, N], F32, tag="den", bufs=2, name="den")
                    nc.scalar.activation(den[:], ctx[64:64 + NONES, :], AF.Copy)
                    rec = spool.tile([NONES, N], F32, tag="rec", name="rec")
                    nc.vector.reciprocal_approx_fast(rec[:], den[:])
                    nc.vector.tensor_tensor(ctxT[off:off + 32, hc, :], ctx[0:32, :],
                                            rec[:], op=ALU.mult)
                    nc.vector.tensor_tensor(ctxT[off + 32:off + 64, hc, :],
                                            ctx[32:64, :], rec[:], op=ALU.mult)
                proj(ctxT, wo_groups, out_evict)

            for l in range(L):
                def wgrp(a, i):
                    return [wp[l, a, i, g] for g in range(2)]

                # ---- self-attention ----
                layernorm(hT)

                def ev_to(dst):
                    return lambda j, ps: nc.scalar.activation(dst[:, j], ps[:], AF.Copy)
                proj(hT, wgrp(0, 0), ev_to(qT))
                proj(hT, wgrp(0, 1), ev_to(kT))
                v_proj(hT, wgrp(0, 2))

                def ev_self_o(j, ps):
                    nc.vector.tensor_tensor(xT[:, j], ps[:], xT[:, j].bitcast(F32),
                                            op=ALU.add)
                attention(qT, kT, True, wgrp(0, 3), ev_self_o)

                # ---- cross-attention ----
                layernorm(hT)
                proj(hT, wgrp(1, 0), ev_to(qT))
                proj(encT, wgrp(1, 1), ev_to(kT))
                v_proj(encT, wgrp(1, 2))

                def ev_cross_o(j, ps):
                    ct = spool.tile([P, N], F32R, tag="ctile", bufs=2, name="ct")
                    nc.scalar.activation(ct[:], ps[:], AF.Copy)
                    nc.sync.dma_start(cpark[:, j], ct[:])
                    nc.vector.tensor_tensor(xT[:, j], ps[:], xT[:, j].bitcast(F32),
                                            op=ALU.add)
                attention(qT, kT, False, wgrp(1, 3), ev_cross_o)

                # ---- FFN (8 f-eighths, double-buffered u) ----
                layernorm(hT)
                for e8 in range(8):
                    # FFN1: produce eighth e8 of u (4 f-chunks)
                    wt = wpool.tile([P, DCH, N], F32R, tag="wt", name="wt")
                    nc.sync.dma_start(wt[:, :, 0:N // 2], w1p[l, e8][:, :, 0:N // 2])
                    nc.sync.dma_start(wt[:, :, N // 2:], w1p[l, e8][:, :, N // 2:])
                    u8 = work.tile([P, 4, N], F32R, tag="uT", bufs=2, name="u8")
                    for jj in range(4):
                        ps = psum.tile([P, N], F32, tag="mm", bufs=2, name="ps_f1")
                        for k in range(DCH):
                            nc.tensor.matmul(ps[:], wt[:, k, jj * P:(jj + 1) * P],
                                             hT[:, k], start=(k == 0),
                                             stop=(k == DCH - 1))
                        nc.scalar.activation(u8[:, jj], ps[:], AF.Relu)
                    # FFN2: partial contraction over this eighth (all 8 e-chunks)
                    wt2 = wpool.tile([P, DCH, 4, P], F32R, tag="wt", name="wt2")
                    nc.sync.dma_start(wt2[:], w2p[l, e8])
                    for j in range(DCH):
                        ps = psum.tile([P, N], F32, tag="mm", bufs=2, name="ps_f2")
                        for k in range(4):
                            nc.tensor.matmul(ps[:], wt2[:, j, k], u8[:, k],
                                             start=(k == 0), stop=(k == 3))
                        if e8 == 0:
                            # x = cross_out + ffn_part0 (reference residual quirk)
                            ct = spool.tile([P, N], F32R, tag="ctile", bufs=2, name="ct2")
                            nc.sync.dma_start(ct[:], cpark[:, j])
                            nc.vector.tensor_tensor(xT[:, j], ps[:],
                                                    ct[:].bitcast(F32), op=ALU.add)
                        else:
                            nc.vector.tensor_tensor(xT[:, j], ps[:],
                                                    xT[:, j].bitcast(F32), op=ALU.add)

            # ---- transpose to [tokens, features] and store ----
            # j-outer so each feature chunk's transposes overlap the last
            # layer's remaining FFN2 evictions (xT[:, j] finalizes per j).
            for j in range(DCH):
                for m in range(MCH):
                    pst = psum.tile([P, N], F32, tag="sc", bufs=4, name="pst")
                    nc.tensor.transpose(pst[:, 0:P],
                                        xT[:, j, m * P:(m + 1) * P].bitcast(F32),
                                        ident[:])
                    tsb = spool.tile([P, P], F32, tag="osb", bufs=2, name="tsb")
                    nc.scalar.activation(tsb[:], pst[:, 0:P], AF.Copy)
                    nc.sync.dma_start(
                        out[m * P:(m + 1) * P, j * P:(j + 1) * P], tsb[:])

            work.release()

    nc.compile()
    return nc


# ----------------------------------------------------------------------------
# host-side packing
# ----------------------------------------------------------------------------

def _pack_T(aT):
    """[1024, C] (feature-major) -> tile image [128, 8, C]."""
    d, c = aT.shape
    return np.ascontiguousarray(aT.reshape(DCH, P, c).transpose(1, 0, 2))


def _pack_proj(w):
    """w [Dout, Din] (as in y = x @ w.T) -> [2, 128, 8, 512] group tile images."""
    wT = w.T  # [Din, Dout]
    return np.stack([_pack_T(wT[:, g * N:(g + 1) * N]) for g in range(2)])


def _prep(inputs):
    dec_inputs = np.asarray(inputs["dec_inputs"])
    self_mask = np.asarray(inputs["self_mask"])
    enc_output = np.asarray(inputs["enc_output"], dtype=np.float32)
    encoder_mask = np.asarray(inputs["encoder_mask"])
    embed = np.asarray(inputs["embed"], dtype=np.float32)
    pe = np.asarray(inputs["pe"], dtype=np.float32)
    wq, wk, wv, wo = (np.asarray(inputs[k], np.float32) for k in ("wq", "wk", "wv", "wo"))
    w1, w2 = np.asarray(inputs["ffn_w1"], np.float32), np.asarray(inputs["ffn_w2"], np.float32)

    # structural assumptions baked into the kernel
    causal_ref = np.triu(np.ones((LD, LD), bool), k=1)
    assert all(np.array_equal(self_mask[b], causal_ref) for b in range(B)), \
        "kernel assumes causal self mask"
    assert not encoder_mask.any(), "kernel assumes no encoder mask"
    for k in ("bq", "bk", "bv", "bo", "ffn_b1", "ffn_b2", "ln_b"):
        assert not np.asarray(inputs[k]).any(), f"kernel assumes zero {k}"
    assert np.all(np.asarray(inputs["ln_g"]) == 1.0), "kernel assumes unit ln gains"

    # shared (weight) arrays
    wp = np.empty((L, 2, 4, 2, P, DCH, N), np.float32)
    for l in range(L):
        for a in range(2):
            for i, w in enumerate((wq, wk, wv, wo)):
                wp[l, a, i] = _pack_proj(w[l, a])
    w1p = np.empty((L, 8, P, DCH, N), np.float32)
    w2p = np.empty((L, 8, P, DCH, 4, P), np.float32)
    for l in range(L):
        w1T = w1[l].T  # [1024, 4096]
        for g in range(8):
            w1p[l, g] = _pack_T(w1T[:, g * N:(g + 1) * N])
        w2T = w2[l].T  # [4096, 1024]
        # [e8, ko, ki, j, e] -> [e8, ki, j, ko, e]
        blk = w2T.reshape(8, 4, P, DCH, P)
        w2p[l] = np.ascontiguousarray(blk.transpose(0, 2, 3, 1, 4))

    pep = _pack_T(pe.T)
    causal01 = (~causal_ref[:P, :P]).astype(np.float32).T.copy()  # keep[tk, tq]
    jscale = np.full((P, P), 1.0 / D, np.float32)
    vones = np.ones((P, MCH, H, NONES), np.float32)

    shared = dict(wp=wp, w1p=w1p, w2p=w2p, pep=pep, causal=causal01,
                  jscale=jscale, vones=vones)
    in_maps = []
    for b in range(B):
        x0 = embed[dec_inputs[b]]          # [512, 1024]
        m = dict(shared)
        m["x0p"] = _pack_T(np.ascontiguousarray(x0.T))
        m["encp"] = _pack_T(np.ascontiguousarray(enc_output[b].T))
        in_maps.append(m)
    return in_maps


def kernel(**inputs):
    if "nc" not in _CACHE:
        _CACHE["nc"] = _build_nc()
    nc = _CACHE["nc"]
    in_maps = _prep(inputs)

    from concourse import bass_utils
    trace = bool(int(os.environ.get("DECODER_TRACE", "0")))
    res = bass_utils.run_bass_kernel_spmd(
        nc, in_maps, core_ids=list(range(B)), trace=trace)
    _CACHE["last_result"] = res
    return np.stack([res.results[b]["out"] for b in range(B)]).astype(np.float32)


# revision 23
# speedup vs baseline: 1.0496x; 1.0110x over previous
"""Trainium2 Bass kernel for nn_Decoder_68289980006849 (3-layer transformer decoder).

Strategy: data-parallel over batch (B=8) across 8 NeuronCores; zero collectives.
Per core, the full decoder runs in "T-layout" [feature(partitions), token(free)]
with float32r matmuls (full PE rate, ~tf32 precision):

  - embeddings gathered on host (input sharding); device applies *sqrt(D) + pe
  - LayerNorm: mean/mean-of-square via ones-matmul on PE -> stats arrive
    pre-broadcast as [128, 512] PSUM tiles; apply with 2 DVE ops/chunk
  - attention: scores computed transposed (scores_T[tk, tq]) so the softmaxed
    matrix feeds the context matmul directly (no transposes); denominators come
    free from ones-columns appended to V; exp on ACT with fused 1/sqrt(dk);
    causal masking via column-slicing + one diagonal-block multiply
  - normalization by softmax denominator fused into the PSUM eviction
  - FFN processed in 4 f-quarters (SBUF); residuals fused into evictions
    (incl. the reference quirk x = cross_out + ffn_out); cross-attn output
    parked in DRAM between its producer and the FFN2 eviction
  - weights pre-transposed and pre-packed on host into exact SBUF tile images
    so every weight DMA is one contiguous 2MB transfer

Self-contained: only stdlib + numpy + the concourse/bass stack on PYTHONPATH.
"""

import os
import numpy as np

import concourse.bass as bass
import concourse.tile as tile
from concourse import bacc, mybir
from concourse.masks import make_identity

# ---- problem constants (hardcoded per contract) ----
B, LD, LE = 8, 512, 512
D, H, DK, F, L, V = 1024, 16, 64, 4096, 3, 32000
M = LD                      # tokens per core
DCH = D // 128              # 8 d-model chunks
FCH = F // 128              # 32 ffn chunks
MCH = M // 128              # 4 token chunks
SQRT_D = 32.0
INV_SQRT_DK = 0.125
EPS = 1e-5
NONES = 32                  # ones-columns appended to V (denominator rows)

P = 128
N = 512
F32 = mybir.dt.float32
F32R = mybir.dt.float32r
AF = mybir.ActivationFunctionType
ALU = mybir.AluOpType

_CACHE = {}


# ----------------------------------------------------------------------------
# Bass program (identical on all 8 cores; data differs via in_maps)
# ----------------------------------------------------------------------------

def _build_nc():
    nc = bacc.Bacc("TRN2", target_bir_lowering=False, debug=False,
                   enable_asserts=False, num_devices=8)

    # inputs (per core)
    x0p = nc.dram_tensor("x0p", [P, DCH, N], F32R, kind="ExternalInput").ap()
    pep = nc.dram_tensor("pep", [P, DCH, N], F32, kind="ExternalInput").ap()
    encp = nc.dram_tensor("encp", [P, DCH, N], F32R, kind="ExternalInput").ap()
    # projection weights, packed: [l, a, i(q,k,v,o), g, 128, 8, 512]
    wp = nc.dram_tensor("wp", [L, 2, 4, 2, P, DCH, N], F32R, kind="ExternalInput").ap()
    w1p = nc.dram_tensor("w1p", [L, 8, P, DCH, N], F32R, kind="ExternalInput").ap()
    # FFN2 weights packed per f-eighth: [128, j(8), ko(4), 128]
    w2p = nc.dram_tensor("w2p", [L, 8, P, DCH, 4, P], F32R, kind="ExternalInput").ap()
    causal = nc.dram_tensor("causal", [P, P], F32, kind="ExternalInput").ap()
    jscale = nc.dram_tensor("jscale", [P, P], F32R, kind="ExternalInput").ap()
    vones = nc.dram_tensor("vones", [P, MCH, H, NONES], F32R, kind="ExternalInput").ap()
    cpark = nc.dram_tensor("cpark", [P, DCH, N], F32R, kind="Internal").ap()
    out = nc.dram_tensor("out", [M, D], F32, kind="ExternalOutput").ap()

    with tile.TileContext(nc) as tc:
        with tc.tile_pool(name="res", bufs=1) as res, \
             tc.tile_pool(name="wpool", bufs=2) as wpool, \
             tc.tile_pool(name="spool", bufs=2) as spool, \
             tc.tile_pool(name="psum", bufs=1, space="PSUM") as psum:

            # ---- resident tiles ----
            xT = res.tile([P, DCH, N], F32R)        # residual stream
            encT = res.tile([P, DCH, N], F32R)      # encoder output (transposed)
            Vst = res.tile([P, MCH, H, 64 + NONES], F32R)  # V + ones columns
            Jsc = res.tile([P, P], F32R)            # all-(1/D) for LN stats
            c01 = res.tile([P, P], F32)             # causal diagonal 0/1 keep-mask
            ident = res.tile([P, P], F32)
            epsc = res.tile([P, 1], F32)

            nc.sync.dma_start(encT[:], encp)
            nc.sync.dma_start(Jsc[:], jscale)
            nc.sync.dma_start(c01[:], causal)
            nc.sync.dma_start(Vst[:, :, :, 64:64 + NONES], vones)
            make_identity(nc, ident[:])
            nc.vector.memset(epsc[:], EPS)

            # ---- x = x0 * sqrt(D) + pe (short-lived staging pool) ----
            with tc.tile_pool(name="init", bufs=1) as init:
                x0 = init.tile([P, DCH, N], F32R, tag="x0")
                pe = init.tile([P, DCH, N], F32, tag="pe")
                nc.sync.dma_start(x0[:], x0p)
                nc.sync.dma_start(pe[:], pep)
                for c in range(DCH):
                    nc.vector.scalar_tensor_tensor(
                        xT[:, c], x0[:, c].bitcast(F32), SQRT_D, pe[:, c],
                        ALU.mult, ALU.add)

            work = tc.alloc_tile_pool(name="work", bufs=1)
            hT = work.tile([P, DCH, N], F32R, tag="hT")
            ctxT = work.tile([P, DCH, N], F32R, tag="ctxT")
            qT = work.tile([P, DCH, N], F32R, tag="qT")
            kT = work.tile([P, DCH, N], F32R, tag="kT")


            # ---- helpers ----
            def layernorm(dst):
                """dst[:] = layernorm(xT) along features."""
                mean_ps = psum.tile([P, N], F32, tag="sc", bufs=4, name="mean_ps")
                msq_ps = psum.tile([P, N], F32, tag="sc", bufs=4, name="msq_ps")
                for c in range(DCH):
                    sq = spool.tile([P, N], F32R, tag="sq", bufs=2, name="sq")
                    nc.scalar.activation(sq[:], xT[:, c].bitcast(F32), AF.Square)
                    nc.tensor.matmul(mean_ps[:], Jsc[:], xT[:, c],
                                     start=(c == 0), stop=(c == DCH - 1))
                    nc.tensor.matmul(msq_ps[:], Jsc[:], sq[:],
                                     start=(c == 0), stop=(c == DCH - 1))
                m2 = spool.tile([P, N], F32, tag="stt", name="m2")
                nc.scalar.activation(m2[:], mean_ps[:], AF.Square)
                var = spool.tile([P, N], F32, tag="stt", name="var")
                nc.vector.tensor_tensor(var[:], msq_ps[:], m2[:], op=ALU.subtract)
                sd = spool.tile([P, N], F32, tag="stt", name="sd")
                nc.scalar.activation(sd[:], var[:], AF.Sqrt, bias=epsc[:])
                rstd = spool.tile([P, N], F32, tag="stt", name="rstd")
                nc.vector.reciprocal_approx_fast(rstd[:], sd[:])
                for c in range(DCH):
                    cen = spool.tile([P, N], F32, tag="cen", name="cen")
                    nc.vector.tensor_tensor(cen[:], xT[:, c].bitcast(F32), mean_ps[:],
                                            op=ALU.subtract)
                    nc.vector.tensor_tensor(dst[:, c], cen[:], rstd[:], op=ALU.mult)

            def proj(rhs, w_groups, evict):
                """out[j] = sum_k W[k, j-chunk].T @ rhs[k]; W streamed in 2MB tiles.
                The DMA lands in column halves so the first out-chunks' matmuls
                can start as soon as half the tile has arrived."""
                for g in range(2):
                    wt = wpool.tile([P, DCH, N], F32R, tag="wt", name="wt")
                    nc.sync.dma_start(wt[:, :, 0:N // 2], w_groups[g][:, :, 0:N // 2])
                    nc.sync.dma_start(wt[:, :, N // 2:], w_groups[g][:, :, N // 2:])
                    for jj in range(4):
                        j = g * 4 + jj
                        ps = psum.tile([P, N], F32, tag="mm", bufs=2, name="ps_mm")
                        for k in range(DCH):
                            nc.tensor.matmul(ps[:], wt[:, k, jj * P:(jj + 1) * P],
                                             rhs[:, k], start=(k == 0),
                                             stop=(k == DCH - 1))
                        evict(j, ps)

            def v_proj(kv_rhs, w_groups):
                """V_nat[tk, dv] -> Vst[:, t, h, :64] slices."""
                for g in range(2):  # dv halves (heads g*8..g*8+7)
                    wt = wpool.tile([P, DCH, N], F32R, tag="wt", name="wt")
                    nc.sync.dma_start(wt[:], w_groups[g])
                    for t in range(MCH):
                        ps = psum.tile([P, N], F32, tag="mm", bufs=2, name="ps_v")
                        for k in range(DCH):
                            nc.tensor.matmul(ps[:], kv_rhs[:, k, t * P:(t + 1) * P],
                                             wt[:, k], start=(k == 0),
                                             stop=(k == DCH - 1))
                        nc.scalar.activation(
                            Vst[:, t, g * 8:(g + 1) * 8, 0:64],
                            ps[:].rearrange("p (h d) -> p h d", d=64), AF.Copy)

            def attention(q_t, k_t, is_self, wo_groups, out_evict):
                """softmax((q_t.T k_t)/sqrt(dk)) @ V via transposed scores.

                Head pairs are issued adjacently: the even head's K=64 score
                matmuls occupy PE rows 0:64, the odd head's rows 64:128
                (tile_position auto-derived from base_partition), so both run
                concurrently in the array."""
                for h in range(H):
                    hc, off = h // 2, 64 * (h % 2)
                    exps = []
                    for c in range(MCH):
                        cs = c * P if is_self else 0
                        sc = psum.tile([P, N], F32, tag="sc", bufs=4, name="sc")
                        nc.tensor.matmul(sc[:, cs:],
                                         k_t[off:off + 64, hc, c * P:(c + 1) * P],
                                         q_t[off:off + 64, hc, cs:],
                                         start=True, stop=True)
                        ex = spool.tile([P, N], F32R, tag="exp", bufs=5, name="ex")
                        nc.scalar.activation(ex[:, cs:], sc[:, cs:], AF.Exp,
                                             scale=INV_SQRT_DK)
                        if is_self:
                            nc.gpsimd.tensor_tensor(
                                ex[:, c * P:(c + 1) * P],
                                ex[:, c * P:(c + 1) * P].bitcast(F32), c01[:],
                                op=ALU.mult)
                        exps.append(ex)
                    ctx = psum.tile([P, N], F32, tag="ctx", bufs=2, name="ctx")
                    for c in range(MCH):
                        cs = c * P if is_self else 0
                        nc.tensor.matmul(ctx[0:64 + NONES, cs:], Vst[:, c, h, :],
                                         exps[c][:, cs:], start=(c == 0),
                                         stop=(c == MCH - 1), skip_group_check=True)
                    den = spool.tile([NONES, N], F32, tag="den", bufs=2, name="den")
                    nc.scalar.activation(den[:], ctx[64:64 + NONES, :], AF.Copy)
                    rec = spool.tile([NONES, N], F32, tag="rec", name="rec")
                    nc.vector.reciprocal_approx_fast(rec[:], den[:])
                    nc.vector.tensor_tensor(ctxT[off:off + 32, hc, :], ctx[0:32, :],
                                            rec[:], op=ALU.mult)
                    nc.vector.tensor_tensor(ctxT[off + 32:off + 64, hc, :],
                                            ctx[32:64, :], rec[:], op=ALU.mult)
                proj(ctxT, wo_groups, out_evict)

            for l in range(L):
                def wgrp(a, i):
                    return [wp[l, a, i, g] for g in range(2)]

                # ---- self-attention ----
                layernorm(hT)

                def ev_to(dst):
                    return lambda j, ps: nc.scalar.activation(dst[:, j], ps[:], AF.Copy)
                proj(hT, wgrp(0, 0), ev_to(qT))
                proj(hT, wgrp(0, 1), ev_to(kT))
                v_proj(hT, wgrp(0, 2))

                def ev_self_o(j, ps):
                    nc.vector.tensor_tensor(xT[:, j], ps[:], xT[:, j].bitcast(F32),
                                            op=ALU.add)
                attention(qT, kT, True, wgrp(0, 3), ev_self_o)

                # ---- cross-attention ----
                layernorm(hT)
                proj(hT, wgrp(1, 0), ev_to(qT))
                proj(encT, wgrp(1, 1), ev_to(kT))
                v_proj(encT, wgrp(1, 2))

                def ev_cross_o(j, ps):
                    ct = spool.tile([P, N], F32R, tag="ctile", bufs=2, name="ct")
                    nc.scalar.activation(ct[:], ps[:], AF.Copy)
                    nc.sync.dma_start(cpark[:, j], ct[:])
                    nc.vector.tensor_tensor(xT[:, j], ps[:], xT[:, j].bitcast(F32),
                                            op=ALU.add)
                attention(qT, kT, False, wgrp(1, 3), ev_cross_o)

                # ---- FFN (8 f-eighths, double-buffered u) ----
                layernorm(hT)
                for e8 in range(8):
                    # FFN1: produce eighth e8 of u (4 f-chunks)
                    wt = wpool.tile([P, DCH, N], F32R, tag="wt", name="wt")
                    nc.sync.dma_start(wt[:, :, 0:N // 2], w1p[l, e8][:, :, 0:N // 2])
                    nc.sync.dma_start(wt[:, :, N // 2:], w1p[l, e8][:, :, N // 2:])
                    u8 = work.tile([P, 4, N], F32R, tag="uT", bufs=2, name="u8")
                    for jj in range(4):
                        ps = psum.tile([P, N], F32, tag="mm", bufs=2, name="ps_f1")
                        for k in range(DCH):
                            nc.tensor.matmul(ps[:], wt[:, k, jj * P:(jj + 1) * P],
                                             hT[:, k], start=(k == 0),
                                             stop=(k == DCH - 1))
                        nc.scalar.activation(u8[:, jj], ps[:], AF.Relu)
                    # FFN2: partial contraction over this eighth (all 8 e-chunks)
                    wt2 = wpool.tile([P, DCH, 4, P], F32R, tag="wt", name="wt2")
                    nc.sync.dma_start(wt2[:], w2p[l, e8])
                    for j in range(DCH):
                        ps = psum.tile([P, N], F32, tag="mm", bufs=2, name="ps_f2")
                        for k in range(4):
                            nc.tensor.matmul(ps[:], wt2[:, j, k], u8[:, k],
                                             start=(k == 0), stop=(k == 3))
                        if e8 == 0:
                            # x = cross_out + ffn_part0 (reference residual quirk)
                            ct = spool.tile([P, N], F32R, tag="ctile", bufs=2, name="ct2")
                            nc.sync.dma_start(ct[:], cpark[:, j])
                            nc.vector.tensor_tensor(xT[:, j], ps[:],
                                                    ct[:].bitcast(F32), op=ALU.add)
                        else:
                            nc.vector.tensor_tensor(xT[:, j], ps[:],
                                                    xT[:, j].bitcast(F32), op=ALU.add)

            # ---- transpose to [tokens, features] and store ----
            # j-outer so each feature chunk's transposes overlap the last
            # layer's remaining FFN2 evictions (xT[:, j] finalizes per j).
            for j in range(DCH):
                for m in range(MCH):
                    pst = psum.tile([P, N], F32, tag="sc", bufs=4, name="pst")
                    nc.tensor.transpose(pst[:, 0:P],
                                        xT[:, j, m * P:(m + 1) * P].bitcast(F32),
                                        ident[:])
                    tsb = spool.tile([P, P], F32, tag="osb", bufs=2, name="tsb")
                    nc.scalar.activation(tsb[:], pst[:, 0:P], AF.Copy)
                    nc.sync.dma_start(
                        out[m * P:(m + 1) * P, j * P:(j + 1) * P], tsb[:])

            work.release()

    nc.compile()
    return nc


# ----------------------------------------------------------------------------
# host-side packing
# ----------------------------------------------------------------------------

def _pack_T(aT):
    """[1024, C] (feature-major) -> tile image [128, 8, C]."""
    d, c = aT.shape
    return np.ascontiguousarray(aT.reshape(DCH, P, c).transpose(1, 0, 2))


def _pack_proj(w):
    """w [Dout, Din] (as in y = x @ w.T) -> [2, 128, 8, 512] group tile images."""
    wT = w.T  # [Din, Dout]
    return np.stack([_pack_T(wT[:, g * N:(g + 1) * N]) for g in range(2)])


def _prep(inputs):
    dec_inputs = np.asarray(inputs["dec_inputs"])
    self_mask = np.asarray(inputs["self_mask"])
    enc_output = np.asarray(inputs["enc_output"], dtype=np.float32)
    encoder_mask = np.asarray(inputs["encoder_mask"])
    embed = np.asarray(inputs["embed"], dtype=np.float32)
    pe = np.asarray(inputs["pe"], dtype=np.float32)
    wq, wk, wv, wo = (np.asarray(inputs[k], np.float32) for k in ("wq", "wk", "wv", "wo"))
    w1, w2 = np.asarray(inputs["ffn_w1"], np.float32), np.asarray(inputs["ffn_w2"], np.float32)

    # structural assumptions baked into the kernel
    causal_ref = np.triu(np.ones((LD, LD), bool), k=1)
    assert all(np.array_equal(self_mask[b], causal_ref) for b in range(B)), \
        "kernel assumes causal self mask"
    assert not encoder_mask.any(), "kernel assumes no encoder mask"
    for k in ("bq", "bk", "bv", "bo", "ffn_b1", "ffn_b2", "ln_b"):
        assert not np.asarray(inputs[k]).any(), f"kernel assumes zero {k}"
    assert np.all(np.asarray(inputs["ln_g"]) == 1.0), "kernel assumes unit ln gains"

    # shared (weight) arrays
    wp = np.empty((L, 2, 4, 2, P, DCH, N), np.float32)
    for l in range(L):
        for a in range(2):
            for i, w in enumerate((wq, wk, wv, wo)):
                wp[l, a, i] = _pack_proj(w[l, a])
    w1p = np.empty((L, 8, P, DCH, N), np.float32)
    w2p = np.empty((L, 8, P, DCH, 4, P), np.float32)
    for l in range(L):
        w1T = w1[l].T  # [1024, 4096]
        for g in range(8):
            w1p[l, g] = _pack_T(w1T[:, g * N:(g + 1) * N])
        w2T = w2[l].T  # [4096, 1024]
        # [e8, ko, ki, j, e] -> [e8, ki, j, ko, e]
        blk = w2T.reshape(8, 4, P, DCH, P)
        w2p[l] = np.ascontiguousarray(blk.transpose(0, 2, 3, 1, 4))

    pep = _pack_T(pe.T)
    causal01 = (~causal_ref[:P, :P]).astype(np.float32).T.copy()  # keep[tk, tq]
    jscale = np.full((P, P), 1.0 / D, np.float32)
    vones = np.ones((P, MCH, H, NONES), np.float32)

    shared = dict(wp=wp, w1p=w1p, w2p=w2p, pep=pep, causal=causal01,
                  jscale=jscale, vones=vones)
    in_maps = []
    for b in range(B):
        x0 = embed[dec_inputs[b]]          # [512, 1024]
        m = dict(shared)
        m["x0p"] = _pack_T(np.ascontiguousarray(x0.T))
        m["encp"] = _pack_T(np.ascontiguousarray(enc_output[b].T))
        in_maps.append(m)
    return in_maps


def kernel(**inputs):
    if "nc" not in _CACHE:
        _CACHE["nc"] = _build_nc()
    nc = _CACHE["nc"]
    in_maps = _prep(inputs)

    from concourse import bass_utils
    trace = bool(int(os.environ.get("DECODER_TRACE", "0")))
    res = bass_utils.run_bass_kernel_spmd(
        nc, in_maps, core_ids=list(range(B)), trace=trace)
    _CACHE["last_result"] = res
    return np.stack([res.results[b]["out"] for b in range(B)]).astype(np.float32)
